# revision 22
# baseline (speedup 1.0000x reference)
"""PixelRNN row-LSTM layer on 8 Trainium2 NeuronCores.

Strategy: data-parallel over batch (B=8 -> 1 image per core). Each core runs
the full H=128 row recurrence for its image:
  gates(r) = W_is_masked * X(r)  (1x3 masked conv, taps dx in {-1,0})
           + W_ss * h(r-1)       (1x3 conv, taps dx in {-1,0,+1})
  f,i,o = sigmoid(...), g = tanh(...), c = f*c + i*g, h = o*tanh(c)

Per-core design notes:
  - All matmuls bf16 (fp32 PSUM accumulation); c state fp16; gates/h bf16.
  - Gate channels permuted host-side to gate-major order, chunk order
    [i, g, f, o]. Each gate chunk has its OWN PSUM tile/bank (tile-granular
    dependency tracking otherwise serializes the per-gate activations
    against all of the row's matmuls). i/g/f double-buffered, o
    single-buffered, leaving one bank for the warm-up dummy.
  - Conv taps realized as column-shifted sub-range matmuls accumulating into
    the same PSUM bank (boundary columns simply don't receive the
    out-of-range tap = correct zero contribution). No padding needed.
  - Bias folded into the center-tap input matmul via an ones-row appended to
    X (K=97); ACT ops then need no bias operand.
  - Zero-consumer "filler" matmuls into a dummy PSUM bank keep the PE HAM
    clock-gate at K=8/8 (2.4 GHz); without them the PE runs at 1.2 GHz the
    whole kernel (measured: 404us -> matmul gaps 107ns vs 56ns warm).
    A dense pre-warm block at kernel start latches the HAM early.
  - Explicit same-engine ordering deps on the ACT and DVE chains; the
    priority-heap scheduler otherwise runs sigma_o before tanh_g and adds
    ~0.5us/row to the critical chain (measured).
"""

import sys
import numpy as np

sys.path.insert(0, "/opt/trn_rl_repo")

import ml_dtypes

BF16 = ml_dtypes.bfloat16

B, C, H, W = 8, 96, 128, 128
O = 96
G4 = 4 * O  # 384
NCORES = 8
NFILL = 6  # PE-warming filler matmuls per row
NPREWARM = 40  # dense filler block before the row loop
# chunk order: k=0:i, 1:g, 2:f, 3:o   (reference gate index: f=0,i=1,o=2,g=3)
CHUNK_GATE = [1, 3, 0, 2]

_prog_cache = {}


def _make_mask_center(c_out, c_in):
    # center-tap mask (mask type 'B'): R out sees R in; G sees R,G; B sees all
    m = np.zeros((c_out, c_in), dtype=np.float32)
    co, ci = c_out // 3, c_in // 3
    m[:co, :ci] = 1.0
    m[co : 2 * co, : 2 * ci] = 1.0
    m[2 * co :, :] = 1.0
    return m


def _prep_weights(W_is, b_is, W_ss, b_ss):
    """Returns w_pack [97, 20*128] bf16.

    Rows 0..95: input-channel weights; row 96: bias (only on is-q1 tiles).
    Tile idx: idx = q*4 + k for ss taps q in 0..2; idx = 12 + q*4 + k for is
    taps q in 0..1, chunks k in 0..3 = [i, g, f, o].
    """
    Wm = W_is[:, :, 0, :].astype(np.float32).copy()  # (384, 96, 3)
    Wm[:, :, 2] = 0.0  # right tap masked
    Wm[:, :, 1] *= _make_mask_center(G4, C)  # center tap block mask
    gmap = np.zeros((4, O), dtype=np.int64)
    for k, gt in enumerate(CHUNK_GATE):
        for color in range(3):
            for j in range(32):
                gmap[k, color * 32 + j] = color * 128 + gt * 32 + j

    w_pack = np.zeros((C + 1, 20 * 128), dtype=np.float32)
    for q in range(3):
        for k in range(4):
            idx = q * 4 + k
            w_pack[:C, idx * 128 : idx * 128 + O] = W_ss[gmap[k], :, q].T
    for q in range(2):
        for k in range(4):
            idx = 12 + q * 4 + k
            w_pack[:C, idx * 128 : idx * 128 + O] = Wm[gmap[k], :, q].T
            if q == 1:  # bias rides the center input tap
                w_pack[C, idx * 128 : idx * 128 + O] = (
                    b_is.astype(np.float32)[gmap[k]] + b_ss.astype(np.float32)[gmap[k]]
                )
    return w_pack.astype(BF16)


def _build_program():
    import concourse.bass as bass
    import concourse.bacc as bacc
    import concourse.tile as tile
    from concourse.tile_rust import add_dep_helper
    from concourse import mybir

    dt = mybir.dt
    AF = mybir.ActivationFunctionType

    nc = bacc.Bacc("TRN2", target_bir_lowering=False, debug=False, num_devices=NCORES)

    # Wpack declared first: inputs stage host->device in declaration order and
    # the weights gate row 0. Xin/Out are row-major [H, ch, W] so each per-row
    # DMA is one contiguous 25KB block (strided 256B packets run ~4x slower).
    w_dram = nc.dram_tensor("Wpack", [C + 1, 20 * 128], dt.bfloat16, kind="ExternalInput")
    x_dram = nc.dram_tensor("Xin", [H, C + 1, W], dt.bfloat16, kind="ExternalInput")
    out_dram = nc.dram_tensor("Out", [H, C, W], dt.bfloat16, kind="ExternalOutput")
    dbg_dram = nc.dram_tensor("Dbg", [1, 2], dt.float32, kind="ExternalOutput")

    XCH = 16  # X rows per DMA chunk

    def chain(insts):
        # add_dep_helper(X, Y) == "X waits on Y"
        for a, b_ in zip(insts, insts[1:]):
            if a is not None and b_ is not None:
                add_dep_helper(b_.ins, a.ins, False, "order")

    with tile.TileContext(nc) as tc:
        with (
            tc.tile_pool(name="consts", bufs=1) as consts,
            tc.tile_pool(name="xbuf", bufs=1) as xbuf,
            tc.tile_pool(name="state", bufs=2) as state,
            tc.tile_pool(name="gates", bufs=2) as gates,
            tc.tile_pool(name="psum", bufs=2, space="PSUM") as psum,
        ):
            h_prev = state.tile([C, W], dt.bfloat16, tag="h", name="h_init")
            nc.vector.memset(h_prev[:], 0.0)
            c_prev = state.tile([C, W], dt.float16, tag="c", name="c_init")
            nc.vector.memset(c_prev[:], 0.0)

            dum = psum.tile([128, 512], dt.float32, tag="dum", bufs=1, name="dum")

            def filler(n, rhs):
                for _ in range(n):
                    nc.tensor.matmul(
                        dum[:, 0 : rhs.shape[-1]],
                        h_prev[:, 0:W],
                        rhs,
                        start=True,
                        stop=True,
                    )

            # pre-warm depends only on h_init: starts during the weight DMA
            filler(NPREWARM, h_prev[:, 0:W])

            # weights: split over the two HWDGE queues; the input-conv slices
            # (cols 1536:2560, needed by row 0's first matmuls) go first
            wt = consts.tile([C + 1, 20 * 128], dt.bfloat16)
            for j, eng in [
                (3, nc.sync),
                (4, nc.scalar),
                (0, nc.sync),
                (1, nc.scalar),
                (2, nc.sync),
            ]:
                eng.dma_start(
                    wt[:, j * 512 : (j + 1) * 512], w_dram[:, j * 512 : (j + 1) * 512]
                )

            def w_ss(q, k):
                i = q * 4 + k
                return wt[0:C, i * 128 : (i + 1) * 128]

            def w_is(q, k, rows):
                i = 12 + q * 4 + k
                return wt[0:rows, i * 128 : (i + 1) * 128]

            # X rows: contiguous per-row DMAs, prefetched 8 rows ahead in-loop
            XPRE = 8
            xrows = []

            def push_x(r):
                xt = xbuf.tile([C + 1, W], dt.bfloat16, tag="xr", bufs=24, name=f"x{r}")
                nc.sync.dma_start(xt[:], x_dram[r])
                xrows.append(xt)

            for r in range(XPRE):
                push_x(r)

            for r in range(H):
                xc = xrows[r]

                pi = psum.tile([128, W], dt.float32, tag="pi", name=f"pi_{r}")
                pg = psum.tile([128, W], dt.float32, tag="pg", name=f"pg_{r}")
                pf = psum.tile([128, W], dt.float32, tag="pf", name=f"pf_{r}")
                po = psum.tile([128, W], dt.float32, tag="po", bufs=1, name=f"po_{r}")
                cols = [pi, pg, pf, po]
                # --- x-part (8 MMs): taps q=1 (center, +bias row), q=0 (left)
                for k in range(4):
                    p = cols[k]
                    nc.tensor.matmul(
                        p[:, 0:W],
                        w_is(1, k, C + 1),
                        xc[0 : C + 1, 0:W],
                        start=True,
                        stop=False,
                    )
                    nc.tensor.matmul(
                        p[:, 1:W],
                        w_is(0, k, C),
                        xc[0:C, 0 : W - 1],
                        start=False,
                        stop=False,
                    )
                # --- h-part (12 MMs): taps q=1 (center), q=0 (left), q=2 (right)
                for k in range(4):
                    p = cols[k]
                    nc.tensor.matmul(
                        p[:, 0:W], w_ss(1, k), h_prev[:, 0:W], start=False, stop=False
                    )
                    nc.tensor.matmul(
                        p[:, 1:W],
                        w_ss(0, k),
                        h_prev[:, 0 : W - 1],
                        start=False,
                        stop=False,
                    )
                    nc.tensor.matmul(
                        p[:, 0 : W - 1],
                        w_ss(2, k),
                        h_prev[:, 1:W],
                        start=False,
                        stop=True,
                    )
                # --- PE-warming fillers (results never used; see module doc)
                filler(NFILL, wt[0:C, 0:512])

                # --- activations (ordering enforced; scheduler misorders)
                i_t = gates.tile([O, W], dt.bfloat16, tag="i", name=f"i_{r}")
                a1 = nc.scalar.activation(i_t[:], pi[0:O, 0:W], AF.Sigmoid)
                g_t = gates.tile([O, W], dt.bfloat16, tag="g", name=f"g_{r}")
                a2 = nc.scalar.activation(g_t[:], pg[0:O, 0:W], AF.Tanh)
                f_t = gates.tile([O, W], dt.bfloat16, tag="f", name=f"f_{r}")
                a3 = nc.scalar.activation(f_t[:], pf[0:O, 0:W], AF.Sigmoid)
                o_t = gates.tile([O, W], dt.bfloat16, tag="o", name=f"o_{r}")
                a4 = nc.scalar.activation(o_t[:], po[0:O, 0:W], AF.Sigmoid)

                # --- c update (DVE, fp16 for the 2x mode)
                ig = gates.tile([O, W], dt.float16, tag="ig", name=f"ig_{r}")
                v1 = nc.vector.tensor_mul(ig[:], i_t[:], g_t[:])
                fc_t = gates.tile([O, W], dt.float16, tag="fc", name=f"fc_{r}")
                v2 = nc.vector.tensor_mul(fc_t[:], f_t[:], c_prev[:])
                c_new = state.tile([C, W], dt.float16, tag="c", name=f"c_{r}")
                v3 = nc.vector.tensor_add(c_new[:], ig[:], fc_t[:])

                tc_t = gates.tile([O, W], dt.bfloat16, tag="tc", name=f"tc_{r}")
                a5 = nc.scalar.activation(tc_t[:], c_new[:], AF.Tanh)
                h_new = state.tile([C, W], dt.bfloat16, tag="h", name=f"h_{r}")
                v4 = nc.vector.tensor_mul(h_new[:], o_t[:], tc_t[:])

                chain([a1, a2, a3, a4, a5])
                chain([v1, v2, v3, v4])

                if r + XPRE < H:
                    push_x(r + XPRE)
                nc.sync.dma_start(out_dram[r], h_new[:])

                h_prev, c_prev = h_new, c_new

            # keep the filler bank observably live
            dbg_t = consts.tile([1, 2], dt.float32)
            nc.vector.tensor_copy(dbg_t[:], dum[0:1, 0:2])
            nc.sync.dma_start(dbg_dram[:], dbg_t[:])

    nc.compile()
    return nc


def _get_program():
    if "nc" not in _prog_cache:
        _prog_cache["nc"] = _build_program()
    return _prog_cache["nc"]


def _prep_x(Xb):
    xa = np.empty((H, C + 1, W), dtype=BF16)
    xa[:, :C] = Xb.transpose(1, 0, 2).astype(BF16)
    xa[:, C] = np.asarray(1.0, dtype=BF16)
    return xa


def _run(inputs, trace=False):
    from concourse.bass_utils import run_bass_kernel_spmd

    X = np.asarray(inputs["X"])
    w_pack = _prep_weights(
        np.asarray(inputs["W_is"]),
        np.asarray(inputs["b_is"]),
        np.asarray(inputs["W_ss"]),
        np.asarray(inputs["b_ss"]),
    )
    nc = _get_program()
    in_maps = [{"Xin": _prep_x(X[b]), "Wpack": w_pack} for b in range(NCORES)]
    res = run_bass_kernel_spmd(nc, in_maps, list(range(NCORES)), trace=trace)
    out = np.stack(
        [
            res.results[b]["Out"].astype(np.float32).transpose(1, 0, 2)
            for b in range(NCORES)
        ],
        axis=0,
    )
    return out, res.exec_time_ns


def kernel(**inputs):
    out, _ = _run(inputs, trace=False)
    return out


# revision 25
# speedup vs baseline: 1.0194x; 1.0194x over previous
"""PixelRNN row-LSTM layer on 8 Trainium2 NeuronCores.

Strategy: data-parallel over batch (B=8 -> 1 image per core). Each core runs
the full H=128 row recurrence for its image:
  gates(r) = W_is_masked * X(r)  (1x3 masked conv, taps dx in {-1,0})
           + W_ss * h(r-1)       (1x3 conv, taps dx in {-1,0,+1})
  f,i,o = sigmoid(...), g = tanh(...), c = f*c + i*g, h = o*tanh(c)

Per-core design notes:
  - All matmuls bf16 (fp32 PSUM accumulation); c state fp16; gates/h bf16.
  - Gate channels permuted host-side to gate-major order, chunk order
    [i, g, f, o]. Each gate chunk has its OWN PSUM tile/bank (tile-granular
    dependency tracking otherwise serializes the per-gate activations
    against all of the row's matmuls). i/g/f double-buffered, o
    single-buffered, leaving one bank for the warm-up dummy.
  - Conv taps realized as column-shifted sub-range matmuls accumulating into
    the same PSUM bank (boundary columns simply don't receive the
    out-of-range tap = correct zero contribution). No padding needed.
  - Bias folded into the center-tap input matmul via an ones-row appended to
    X (K=97); ACT ops then need no bias operand.
  - Zero-consumer "filler" matmuls into a dummy PSUM bank keep the PE HAM
    clock-gate at K=8/8 (2.4 GHz); without them the PE runs at 1.2 GHz the
    whole kernel (measured: 404us -> matmul gaps 107ns vs 56ns warm).
    A dense pre-warm block at kernel start latches the HAM early.
  - Explicit same-engine ordering deps on the ACT and DVE chains; the
    priority-heap scheduler otherwise runs sigma_o before tanh_g and adds
    ~0.5us/row to the critical chain (measured).
"""

import sys
import numpy as np

sys.path.insert(0, "/opt/trn_rl_repo")

import ml_dtypes

BF16 = ml_dtypes.bfloat16

B, C, H, W = 8, 96, 128, 128
O = 96
G4 = 4 * O  # 384
NCORES = 8
NFILL = 6  # PE-warming filler matmuls per row
NPREWARM = 40  # dense filler block before the row loop
# chunk order: k=0:i, 1:g, 2:f, 3:o   (reference gate index: f=0,i=1,o=2,g=3)
CHUNK_GATE = [1, 3, 0, 2]

_prog_cache = {}


def _make_mask_center(c_out, c_in):
    # center-tap mask (mask type 'B'): R out sees R in; G sees R,G; B sees all
    m = np.zeros((c_out, c_in), dtype=np.float32)
    co, ci = c_out // 3, c_in // 3
    m[:co, :ci] = 1.0
    m[co : 2 * co, : 2 * ci] = 1.0
    m[2 * co :, :] = 1.0
    return m


def _prep_weights(W_is, b_is, W_ss, b_ss):
    """Returns w_pack [97, 20*128] bf16.

    Rows 0..95: input-channel weights; row 96: bias (only on is-q1 tiles).
    Tile idx: idx = q*4 + k for ss taps q in 0..2; idx = 12 + q*4 + k for is
    taps q in 0..1, chunks k in 0..3 = [i, g, f, o].
    """
    Wm = W_is[:, :, 0, :].astype(np.float32).copy()  # (384, 96, 3)
    Wm[:, :, 2] = 0.0  # right tap masked
    Wm[:, :, 1] *= _make_mask_center(G4, C)  # center tap block mask
    gmap = np.zeros((4, O), dtype=np.int64)
    for k, gt in enumerate(CHUNK_GATE):
        for color in range(3):
            for j in range(32):
                gmap[k, color * 32 + j] = color * 128 + gt * 32 + j

    w_pack = np.zeros((C + 1, 20 * 128), dtype=np.float32)
    for q in range(3):
        for k in range(4):
            idx = q * 4 + k
            w_pack[:C, idx * 128 : idx * 128 + O] = W_ss[gmap[k], :, q].T
    for q in range(2):
        for k in range(4):
            idx = 12 + q * 4 + k
            w_pack[:C, idx * 128 : idx * 128 + O] = Wm[gmap[k], :, q].T
            if q == 1:  # bias rides the center input tap
                w_pack[C, idx * 128 : idx * 128 + O] = (
                    b_is.astype(np.float32)[gmap[k]] + b_ss.astype(np.float32)[gmap[k]]
                )
    return w_pack.astype(BF16)


def _build_program():
    import concourse.bass as bass
    import concourse.bacc as bacc
    import concourse.tile as tile
    from concourse.tile_rust import add_dep_helper
    from concourse import mybir

    dt = mybir.dt
    AF = mybir.ActivationFunctionType

    nc = bacc.Bacc("TRN2", target_bir_lowering=False, debug=False, num_devices=NCORES)

    # Wpack declared first: inputs stage host->device in declaration order and
    # the weights gate row 0. Xin/Out are row-major [H, ch, W] so each per-row
    # DMA is one contiguous 25KB block (strided 256B packets run ~4x slower).
    w_dram = nc.dram_tensor("Wpack", [C + 1, 20 * 128], dt.bfloat16, kind="ExternalInput")
    x_dram = nc.dram_tensor("Xin", [H, C + 1, W], dt.bfloat16, kind="ExternalInput")
    out_dram = nc.dram_tensor("Out", [H, C, W], dt.bfloat16, kind="ExternalOutput")
    dbg_dram = nc.dram_tensor("Dbg", [1, 2], dt.float32, kind="ExternalOutput")

    XCH = 16  # X rows per DMA chunk

    def chain(insts):
        # add_dep_helper(X, Y) == "X waits on Y"
        for a, b_ in zip(insts, insts[1:]):
            if a is not None and b_ is not None:
                add_dep_helper(b_.ins, a.ins, False, "order")

    with tile.TileContext(nc) as tc:
        with (
            tc.tile_pool(name="consts", bufs=1) as consts,
            tc.tile_pool(name="xbuf", bufs=1) as xbuf,
            tc.tile_pool(name="state", bufs=2) as state,
            tc.tile_pool(name="gates", bufs=2) as gates,
            tc.tile_pool(name="psum", bufs=2, space="PSUM") as psum,
        ):
            h_prev = state.tile([C, W], dt.bfloat16, tag="h", name="h_init")
            nc.vector.memset(h_prev[:], 0.0)
            c_prev = state.tile([C, W], dt.float16, tag="c", name="c_init")
            nc.vector.memset(c_prev[:], 0.0)

            dum = psum.tile([128, 512], dt.float32, tag="dum", bufs=1, name="dum")

            def filler(n, rhs):
                for _ in range(n):
                    nc.tensor.matmul(
                        dum[:, 0 : rhs.shape[-1]],
                        h_prev[:, 0:W],
                        rhs,
                        start=True,
                        stop=True,
                    )

            # pre-warm depends only on h_init: starts during the weight DMA
            filler(NPREWARM, h_prev[:, 0:W])

            # weights: per-queue SBUF-write rate is only ~14GB/s, so spread
            # slices over all three DMA paths; the input-conv slices
            # (cols 1536:2560, needed by row 0's first matmuls) go first
            wt = consts.tile([C + 1, 20 * 128], dt.bfloat16)
            for c0, c1, eng in [
                (1536, 2048, nc.sync),
                (2048, 2560, nc.scalar),
                (0, 384, nc.gpsimd),
                (384, 768, nc.sync),
                (768, 1152, nc.scalar),
                (1152, 1536, nc.gpsimd),
            ]:
                eng.dma_start(wt[:, c0:c1], w_dram[:, c0:c1])

            def w_ss(q, k):
                i = q * 4 + k
                return wt[0:C, i * 128 : (i + 1) * 128]

            def w_is(q, k, rows):
                i = 12 + q * 4 + k
                return wt[0:rows, i * 128 : (i + 1) * 128]

            # X rows: contiguous per-row DMAs on the scalar HWDGE queue (the
            # sync queue is dedicated to output rows), prefetched ahead in-loop
            XPRE = 12
            xrows = []

            def push_x(r, eng=None):
                xt = xbuf.tile([C + 1, W], dt.bfloat16, tag="xr", bufs=24, name=f"x{r}")
                (eng or nc.scalar).dma_start(xt[:], x_dram[r])
                xrows.append(xt)

            # prologue prefetch: split so row 0's activations aren't queued
            # behind a dozen descriptor pushes on the scalar engine
            for r in range(XPRE):
                push_x(r, eng=(nc.scalar if r < 6 else nc.gpsimd))

            for r in range(H):
                xc = xrows[r]

                pi = psum.tile([128, W], dt.float32, tag="pi", name=f"pi_{r}")
                pg = psum.tile([128, W], dt.float32, tag="pg", name=f"pg_{r}")
                pf = psum.tile([128, W], dt.float32, tag="pf", name=f"pf_{r}")
                po = psum.tile([128, W], dt.float32, tag="po", bufs=1, name=f"po_{r}")
                cols = [pi, pg, pf, po]
                # --- x-part (8 MMs): taps q=1 (center, +bias row), q=0 (left)
                for k in range(4):
                    p = cols[k]
                    nc.tensor.matmul(
                        p[:, 0:W],
                        w_is(1, k, C + 1),
                        xc[0 : C + 1, 0:W],
                        start=True,
                        stop=False,
                    )
                    nc.tensor.matmul(
                        p[:, 1:W],
                        w_is(0, k, C),
                        xc[0:C, 0 : W - 1],
                        start=False,
                        stop=False,
                    )
                # --- h-part (12 MMs): taps q=1 (center), q=0 (left), q=2 (right)
                for k in range(4):
                    p = cols[k]
                    nc.tensor.matmul(
                        p[:, 0:W], w_ss(1, k), h_prev[:, 0:W], start=False, stop=False
                    )
                    nc.tensor.matmul(
                        p[:, 1:W],
                        w_ss(0, k),
                        h_prev[:, 0 : W - 1],
                        start=False,
                        stop=False,
                    )
                    nc.tensor.matmul(
                        p[:, 0 : W - 1],
                        w_ss(2, k),
                        h_prev[:, 1:W],
                        start=False,
                        stop=True,
                    )
                # --- PE-warming fillers (results never used; see module doc)
                filler(NFILL, wt[0:C, 0:512])

                # --- activations (ordering enforced; scheduler misorders)
                i_t = gates.tile([O, W], dt.bfloat16, tag="i", name=f"i_{r}")
                a1 = nc.scalar.activation(i_t[:], pi[0:O, 0:W], AF.Sigmoid)
                g_t = gates.tile([O, W], dt.bfloat16, tag="g", name=f"g_{r}")
                a2 = nc.scalar.activation(g_t[:], pg[0:O, 0:W], AF.Tanh)
                f_t = gates.tile([O, W], dt.bfloat16, tag="f", name=f"f_{r}")
                a3 = nc.scalar.activation(f_t[:], pf[0:O, 0:W], AF.Sigmoid)
                o_t = gates.tile([O, W], dt.bfloat16, tag="o", name=f"o_{r}")
                a4 = nc.scalar.activation(o_t[:], po[0:O, 0:W], AF.Sigmoid)

                # --- c update (DVE, fp16 for the 2x mode)
                ig = gates.tile([O, W], dt.float16, tag="ig", name=f"ig_{r}")
                v1 = nc.vector.tensor_mul(ig[:], i_t[:], g_t[:])
                fc_t = gates.tile([O, W], dt.float16, tag="fc", name=f"fc_{r}")
                v2 = nc.vector.tensor_mul(fc_t[:], f_t[:], c_prev[:])
                c_new = state.tile([C, W], dt.float16, tag="c", name=f"c_{r}")
                v3 = nc.vector.tensor_add(c_new[:], ig[:], fc_t[:])

                tc_t = gates.tile([O, W], dt.bfloat16, tag="tc", name=f"tc_{r}")
                a5 = nc.scalar.activation(tc_t[:], c_new[:], AF.Tanh)
                h_new = state.tile([C, W], dt.bfloat16, tag="h", name=f"h_{r}")
                v4 = nc.vector.tensor_mul(h_new[:], o_t[:], tc_t[:])

                chain([a1, a2, a3, a4, a5])
                chain([v1, v2, v3, v4])

                if r + XPRE < H:
                    push_x(r + XPRE)
                nc.sync.dma_start(out_dram[r], h_new[:])

                h_prev, c_prev = h_new, c_new

            # keep the filler bank observably live
            dbg_t = consts.tile([1, 2], dt.float32)
            nc.vector.tensor_copy(dbg_t[:], dum[0:1, 0:2])
            nc.sync.dma_start(dbg_dram[:], dbg_t[:])

    nc.compile()
    return nc


def _get_program():
    if "nc" not in _prog_cache:
        _prog_cache["nc"] = _build_program()
    return _prog_cache["nc"]


def _prep_x(Xb):
    xa = np.empty((H, C + 1, W), dtype=BF16)
    xa[:, :C] = Xb.transpose(1, 0, 2).astype(BF16)
    xa[:, C] = np.asarray(1.0, dtype=BF16)
    return xa


def _run(inputs, trace=False):
    from concourse.bass_utils import run_bass_kernel_spmd

    X = np.asarray(inputs["X"])
    w_pack = _prep_weights(
        np.asarray(inputs["W_is"]),
        np.asarray(inputs["b_is"]),
        np.asarray(inputs["W_ss"]),
        np.asarray(inputs["b_ss"]),
    )
    nc = _get_program()
    in_maps = [{"Xin": _prep_x(X[b]), "Wpack": w_pack} for b in range(NCORES)]
    res = run_bass_kernel_spmd(nc, in_maps, list(range(NCORES)), trace=trace)
    out = np.stack(
        [
            res.results[b]["Out"].astype(np.float32).transpose(1, 0, 2)
            for b in range(NCORES)
        ],
        axis=0,
    )
    return out, res.exec_time_ns


def kernel(**inputs):
    out, _ = _run(inputs, trace=False)
    return out
